# revision 11
# baseline (speedup 1.0000x reference)
"""LID detector kernel v5 for Trainium2 (8 NeuronCores, data-parallel batch).

v5 over v2: DVE pooling units use a single scalar_tensor_tensor with
accum_out per channel column (fold halves + free-dim sum in one op)
for even-width layers, replacing the pairwise fold trees; L1 tail
columns rebalanced from Act to the cheaper DVE path.


Layout per core (batch shard of 32 samples):
  - feat{l} DRAM [32, C, HW] fp8e4m3; DMA'd fully contiguous: partition
    p = 32*w + s holds channels [Cp*w : Cp*w+Cp] of sample s (Cp = C/4).
  - pooling: segment sums over HW via per-channel activation accumulate
    (Act), pairwise-fold trees (DVE), single tensor_reduce (Pool/gpsimd),
    into red2[l] [128, Cp] f32.
  - stream-transpose red2 -> pieces -> qT chunks [Cc, 32] bf16 (qT = sum
    over HW, i.e. HW*q).
  - distances, scaled by HW/2: ps_d[32l+s, j] =
      (HW*q).r + ones x rn2c  where rn2c = -(HW/2)(||r||^2 - C)
    accumulated on PE; d2~ = (HW/2)qn2 + (HW/2)C - ps_d via the Ln bias.
    (rn2c rows are host-precomputed from the bf16 ref tables, like the
    host-side refT transpose: one-time prep of the replicated constants.)
  - top-24 largest of ps_d (= smallest d2) straight from PSUM via
    3x max8 + 2x match_replace.
  - LID = -2k / sum(ln d2_i - ln d2_k)  (scale invariant);
    logit = w.lid + b -> sigmoid.
"""

import os
import sys

for _p in ("/opt/trn_rl_repo", "/root/.axon_site/_ro/trn_rl_repo"):
    if _p not in sys.path:
        sys.path.append(_p)

import numpy as np

import concourse.mybir as mybir
from concourse import bass, bacc
from concourse.dt import dt as cdt
from concourse.tile import TileContext
from concourse.bass_utils import run_bass_kernel_spmd

F32 = mybir.dt.float32
BF16 = mybir.dt.bfloat16
F8 = mybir.dt.float8e4
N_CORES = 8
B = 32
R = 2000
K = 20
LAYERS = [(64, 3136), (128, 784), (256, 196), (512, 49)]
NEG_BIG = -3.0e38
ADD = mybir.AluOpType.add
MULT = mybir.AluOpType.mult

NP_F8 = cdt.np(F8)
NP_BF16 = cdt.np(BF16)


def _rt_chunks(l):
    C = LAYERS[l][0]
    return [(i, min(128, C - i)) for i in range(0, C, 128)]


def _mm_chunks(l):
    # L0 is stored padded to 128 partitions (16 valid rows per 32-block,
    # zeros elsewhere), so its matmul contracts over all 128.
    return [(0, 128)] if l == 0 else _rt_chunks(l)


def _emit_tree(eng, v, scratch, nch, W, out_ap, use_stt=False):
    """Segment-sum v [P, nch, W] (fp8) into out_ap [P, nch] (f32) via
    pairwise folds through scratch [P, nch, W//2] (bf16).

    use_stt: emit scalar_tensor_tensor (a*1+b) instead of tensor_tensor —
    on gpsimd this avoids the low "Add" software-efficiency path."""
    def add(o, a, b):
        if use_stt:
            eng.scalar_tensor_tensor(o, a, 1.0, b, op0=MULT, op1=ADD)
        else:
            eng.tensor_tensor(o, a, b, op=ADD)

    h = W // 2
    add(scratch[:, :, 0:h], v[:, :, 0:h], v[:, :, h:2 * h])
    if W % 2:
        add(scratch[:, :, 0:1], scratch[:, :, 0:1], v[:, :, 2 * h:2 * h + 1])
    W = h
    while W > 2:
        if W % 2:
            add(scratch[:, :, 0:1], scratch[:, :, 0:1], scratch[:, :, W - 1:W])
            W -= 1
            continue
        h = W // 2
        add(scratch[:, :, 0:h], scratch[:, :, 0:h], scratch[:, :, h:W])
        W = h
    add(out_ap, scratch[:, :, 0], scratch[:, :, 1])


# stream plan: ordered events.
#  ("wb",)                         weights + rn2c DMAs + rn2 matmuls
#  ("rt", l)                       ref chunk DMAs for layer l
#  ("feat", l, c0, nch, splits)    feat chunk DMA + pooling units
#       splits: list of (eng, c0, nch) covering [c0, c0+nch)
#  ("finish", l[, half])           transpose + pieces + qsq/qn2 + qT matmuls
PLAN = [
    ("feat", 1, 0, 8, [("dve", 0, 8)]),
    ("feat", 0, 0, 2, [("act", 0, 2)]),
    ("wb",),
    ("feat", 0, 2, 2, [("act", 2, 2)]),
    ("feat", 1, 8, 8, [("dve", 8, 8)]),
    ("feat", 0, 4, 4, [("act", 4, 4)]),
    ("feat", 2, 0, 32, [("pool", 0, 32)]),
    ("feat", 3, 0, 128, [("dve", 0, 64), ("dve", 64, 64)]),
    ("rt", 3),
    ("finish", 3),
    ("feat", 0, 8, 4, [("act", 8, 2), ("dve", 10, 2)]),
    ("feat", 1, 16, 8, [("dve", 16, 5), ("act", 21, 3)]),
    ("feat", 2, 32, 16, [("pool", 32, 16)]),
    ("feat", 2, 48, 16, [("pool", 48, 16)]),
    ("feat", 0, 12, 4, [("dve", 12, 2), ("dve", 14, 2)]),
    ("feat", 1, 24, 8, [("dve", 24, 4), ("act", 28, 4)]),
    ("rt", 2),
    ("rt", 0),
    ("rt", 1),
    ("warm",),
    ("finish", 2),
    ("finish", 0),
    ("finish", 1),
]


def build_nc():
    nc = bacc.Bacc("TRN2", target_bir_lowering=False, debug=False,
                   num_devices=N_CORES)

    feats = [nc.dram_tensor(f"feat{l}", [B, C, HW], F8, kind="ExternalInput")
             for l, (C, HW) in enumerate(LAYERS)]
    refTs = [nc.dram_tensor(f"refT{l}", [128, max(1, C // 128), R], F8,
                            kind="ExternalInput")
             for l, (C, _) in enumerate(LAYERS)]
    rn2c_d = nc.dram_tensor("rn2c", [1, 4 * R], BF16, kind="ExternalInput")
    wb = nc.dram_tensor("wb", [1, 5], F32, kind="ExternalInput")
    wmask_d = nc.dram_tensor("wmask", [128, B], BF16, kind="ExternalInput")
    out = nc.dram_tensor("out", [B, 1], F32, kind="ExternalOutput")
    _dbg = os.environ.get("DEBUG_LID") == "1"
    if _dbg:
        dbg_red = [nc.dram_tensor(f"dbg_red{l}", [128, C // 4], F32,
                                  kind="ExternalOutput")
                   for l, (C, _) in enumerate(LAYERS)]
        dbg_qt = nc.dram_tensor("dbg_qt1", [128, B], F32, kind="ExternalOutput")
        dbg_vals = nc.dram_tensor("dbg_vals", [128, 24], F32,
                                  kind="ExternalOutput")
        dbg_lid = nc.dram_tensor("dbg_lid", [128, 1], BF16, kind="ExternalOutput")
        dbg_qn2 = nc.dram_tensor("dbg_qn2", [128, 1], F32, kind="ExternalOutput")

    with TileContext(nc) as tc:
        with (
            tc.tile_pool(name="persist", bufs=1) as pp,
            tc.tile_pool(name="sq", bufs=2) as sqp,
            tc.tile_pool(name="scr_d", bufs=2) as scrd,
            tc.tile_pool(name="scr_p", bufs=2) as scrp,
            tc.tile_pool(name="tr", bufs=2) as trp,
            tc.tile_pool(name="psum", bufs=1, space=bass.MemorySpace.PSUM) as ps,
        ):
            # ---- persistent tiles
            ft = [pp.tile([128, C // 4, HW], F8, tag=f"ft{l}", name=f"ft{l}")
                  for l, (C, HW) in enumerate(LAYERS)]
            rtbig = [pp.tile([128, max(1, C // 128), R], F8, tag=f"rt{l}",
                             name=f"rt{l}")
                     for l, (C, _) in enumerate(LAYERS)]
            rt = {}
            for l, (C, _) in enumerate(LAYERS):
                for ci, (i, Cc) in enumerate(_mm_chunks(l)):
                    rt[(l, i)] = rtbig[l][:, ci, :]
            rn2c = pp.tile([1, 4 * R], BF16, tag="rn2c", name="rn2c")
            red2 = [pp.tile([128, max(32, C // 4)], F32, tag=f"red2_{l}",
                            name=f"red2_{l}")
                    for l, (C, _) in enumerate(LAYERS)]
            qT = {}
            for l, (C, _) in enumerate(LAYERS):
                for i, Cc in _mm_chunks(l):
                    qT[(l, i)] = pp.tile([Cc, B], BF16, tag=f"qT{l}_{i}",
                                         name=f"qT{l}_{i}")
            act_scratch = pp.tile([128, 3136], BF16, tag="act_scr",
                                  name="act_scr")
            ones1 = pp.tile([1, B], BF16, tag="ones1", name="ones1")
            ones_col = pp.tile([128, 1], BF16, tag="ones_col", name="ones_col")
            ones_row = pp.tile([1, B], F32, tag="ones_row", name="ones_row")
            wb_sb = pp.tile([1, 5], F32, tag="wb_sb", name="wb_sb")
            qn2_all = pp.tile([128, 1], F32, tag="qn2_all", name="qn2_all")
            vals = pp.tile([128, 24], F32, tag="vals", name="vals")
            wbc = pp.tile([B, 5], F32, tag="wbc", name="wbc")
            wmask = pp.tile([128, B], BF16, tag="wmask", name="wmask")
            warm = pp.tile([1, 2], F32, tag="warm", name="warm")

            # PSUM
            ps_d = ps.tile([128, R], F32, tag="ps_d", name="ps_d")
            warm_ps = ps.tile([16, 512], F32, tag="warm_ps", name="warm_ps")
            zeros16 = pp.tile([1, 16], BF16, tag="zeros16", name="zeros16")
            qn2ps = ps.tile([128, 1], F32, tag="qn2ps", name="qn2ps")
            wps = ps.tile([B, 5], F32, tag="wps", name="wps")

            nc.vector.memset(ones1[:], 1.0)
            nc.vector.memset(zeros16[:], 0.0)
            nc.vector.memset(ones_col[:], 1.0)
            nc.vector.memset(ones_row[:], 1.0)
            nc.vector.memset(red2[0][:, 16:32], 0.0)

            BLKS = [(b0, min(512, R - b0)) for b0 in range(0, R, 512)]
            nch_chain = {}    # (l, b0) -> matmuls remaining until stop
            qsq_pending = []  # layers whose qn2 work is deferred past fins

            def emit_wb():
                # pre-load activation table set 6 (ln+exp+copy+square): the
                # whole pipeline stays in one set, no loads on the tail
                nc.scalar.add_instruction(mybir.InstLoadActFuncSet(
                    name=nc.get_next_instruction_name(), ins=[], outs=[],
                    act_func_set_id=6))
                nc.sync.dma_start(out=wb_sb[:], in_=wb[:])
                nc.sync.dma_start(out=wmask[:], in_=wmask_d[:])
                nc.sync.dma_start(out=rn2c[:], in_=rn2c_d[:])
                # broadcast w/b to all partitions right away (tail shortcut)
                nc.tensor.matmul(wps[:], ones_row[:], wb_sb[:],
                                 start=True, stop=True)
                nc.scalar.copy(wbc[:], wps[:])
                # open every accumulation chain with the rn2c row
                for l in range(4):
                    nq = len(_mm_chunks(l))
                    for b0, nb in BLKS:
                        nch_chain[(l, b0)] = nq
                        nc.tensor.matmul(
                            ps_d[32 * l:32 * l + 32, b0:b0 + nb],
                            ones1[:], rn2c[0:1, l * R + b0:l * R + b0 + nb],
                            start=True, stop=False,
                            tile_position=(0, 32 * l),
                            skip_group_check=True)

            def emit_rt(l):
                nc.sync.dma_start(out=rtbig[l][:], in_=refTs[l][:])

            def emit_pool_unit(l, eng_name, c0, nch):
                C, HW = LAYERS[l]
                v = ft[l][:, c0:c0 + nch, :]
                dst = red2[l][:, c0:c0 + nch]
                if eng_name == "act":
                    for j in range(nch):
                        nc.scalar.activation(
                            act_scratch[:, 0:HW], ft[l][:, c0 + j, :],
                            mybir.ActivationFunctionType.Copy,
                            accum_out=red2[l][:, c0 + j:c0 + j + 1])
                elif eng_name == "pool":
                    scratch = scrp.tile([128, nch, HW // 2], BF16,
                                        tag="scr_pool",
                                        name=f"scr_pool_{l}_{c0}")
                    _emit_tree(nc.gpsimd, v, scratch, nch, HW, dst)
                elif HW % 2 == 0:
                    # one stt-accum per column: fold halves + free-dim sum
                    # in a single DVE op (validated on walrus)
                    h = HW // 2
                    for j in range(nch):
                        scr = scrd.tile([128, h], BF16, tag="scr_dve",
                                        name=f"scr_dve_{l}_{c0 + j}")
                        nc.vector.scalar_tensor_tensor(
                            scr[:, 0:h], ft[l][:, c0 + j, 0:h], 1.0,
                            ft[l][:, c0 + j, h:2 * h], op0=MULT, op1=ADD,
                            accum_out=red2[l][:, c0 + j:c0 + j + 1])
                else:
                    scratch = scrd.tile([128, nch, HW // 2], BF16,
                                        tag="scr_dve",
                                        name=f"scr_dve_{l}_{c0}")
                    _emit_tree(nc.vector, v, scratch, nch, HW, dst)

            def emit_qT_mms(l, ksub=None):
                chunks = _mm_chunks(l)
                for ti, (i, Cc) in enumerate(chunks):
                    k0, k1 = (0, Cc) if ksub is None else ksub
                    for b0, nb in BLKS:
                        nch_chain[(l, b0)] -= 1
                        nc.tensor.matmul(
                            ps_d[32 * l:32 * l + 32, b0:b0 + nb],
                            qT[(l, i)][k0:k1, :],
                            rt[(l, i)][k0:k1, b0:b0 + nb],
                            start=False, stop=(nch_chain[(l, b0)] == 0),
                            tile_position=(k0, 32 * l),
                            skip_group_check=True)

            def emit_qsq(l, i, k0, k1, first, last):
                qsq = sqp.tile([128, B], BF16, tag="qsq", name="qsq")
                C, HW = LAYERS[l]
                # qT holds HW*q; want (HW/2)*q^2 = Square(qT/sqrt(2 HW))
                n = k1 - k0
                nc.scalar.activation(qsq[0:n, :], qT[(l, i)][k0:k1, :],
                                     mybir.ActivationFunctionType.Square,
                                     scale=float(1.0 / np.sqrt(2.0 * HW)))
                nc.tensor.matmul(qn2ps[32 * l:32 * l + 32, 0:1],
                                 qsq[0:n, :], ones_col[0:n, 0:1],
                                 start=first, stop=last,
                                 tile_position=(0, 32 * l),
                                 skip_group_check=True)
                if last:
                    nc.scalar.activation(qn2_all[32 * l:32 * l + 32, :],
                                         qn2ps[32 * l:32 * l + 32, :],
                                         mybir.ActivationFunctionType.Copy,
                                         bias=float(HW / 2.0 * C))

            def emit_finish(l, half=None):
                C, HW = LAYERS[l]
                Cp = C // 4
                cols = (0, max(32, Cp))
                w_ = cols[1] - cols[0]
                tr = trp.tile([128, 128], F32, tag="tr", name=f"tr{l}{half}")
                nc.vector.transpose(tr[:, 0:w_], red2[l][:, cols[0]:cols[1]])
                if l == 0:
                    nc.vector.tensor_copy(qT[(0, 0)][:], tr[:, 0:32])
                else:
                    for w in range(4):
                        for v in range(w_ // 32):
                            cglob = Cp * w + cols[0] + 32 * v
                            if cglob >= C:
                                continue
                            i = (cglob // 128) * 128
                            row = cglob - i
                            dst = qT[(l, i)][row:row + 32, :]
                            piece = tr[32 * w:32 * w + 32, 32 * v:32 * v + 32]
                            nc.vector.tensor_copy(dst, piece)
                emit_qT_mms(l)
                qsq_pending.append(l)

            def emit_warm():
                # hold PE p-state through the close cascade: chained dummy
                # matmuls (zero weights) gated on the late rt1 DMA
                for i in range(14):
                    nc.tensor.matmul(warm_ps[0:16, :],
                                     zeros16[0:1, 0:16],
                                     rtbig[1][0:1, 0, 0:512],
                                     start=True, stop=True,
                                     tile_position=(0, 0),
                                     skip_group_check=True)

            for ev in PLAN:
                if ev[0] == "warm":
                    emit_warm()
                elif ev[0] == "wb":
                    emit_wb()
                elif ev[0] == "rt":
                    emit_rt(ev[1])
                elif ev[0] == "feat":
                    _, l, c0, nch, splits = ev
                    C, HW = LAYERS[l]
                    Cp = C // 4
                    src = bass.AP(
                        feats[l], c0 * HW,
                        [[Cp * HW, 4], [C * HW, B], [1, nch * HW]])
                    nc.sync.dma_start(out=ft[l][:, c0:c0 + nch, :], in_=src)
                    for eng_name, ec0, ench in splits:
                        emit_pool_unit(l, eng_name, ec0, ench)
                elif ev[0] == "finish":
                    emit_finish(ev[1], ev[2] if len(ev) > 2 else None)

            # deferred qn2 chains: overlap with topk (only the Ln needs them)
            for l in qsq_pending:
                chunks = _mm_chunks(l)
                for ti, (i, Cc) in enumerate(chunks):
                    emit_qsq(l, i, 0, Cc, ti == 0, ti == len(chunks) - 1)

            if _dbg:
                for l in range(4):
                    C = LAYERS[l][0]
                    nc.sync.dma_start(out=dbg_red[l][:],
                                      in_=red2[l][:, 0:C // 4])
                dq = pp.tile([128, B], F32, tag="dq", name="dq")
                nc.vector.tensor_copy(dq[:], qT[(1, 0)][:])
                nc.sync.dma_start(out=dbg_qt[:], in_=dq[:])

            # ---- top-24 (largest ps_d == smallest d2), straight from PSUM
            ln24 = pp.tile([128, 24], F32, tag="ln24", name="ln24")
            s1 = pp.tile([128, 1], F32, tag="s1", name="s1")
            s2 = pp.tile([128, 1], F32, tag="s2", name="s2")
            nc.vector.max(vals[:, 0:8], ps_d[:])
            nc.vector.match_replace(ps_d[:], vals[:, 0:8], ps_d[:], NEG_BIG)
            nc.vector.max(vals[:, 8:16], ps_d[:])
            # ln of ranks 1..15 overlaps the remaining top-k passes (Act
            # idle); accum_out folds the partial S-reduction in for free
            nc.scalar.activation(ln24[:, 1:16], vals[:, 1:16],
                                 mybir.ActivationFunctionType.Ln,
                                 scale=-1.0, bias=qn2_all[:],
                                 accum_out=s1[:])
            nc.vector.match_replace(ps_d[:], vals[:, 8:16], ps_d[:], NEG_BIG)
            nc.vector.max(vals[:, 16:24], ps_d[:])

            # ---- LID: d2~ = (HW/2)qn2 + (HW/2)C - vals; scale cancels in LID
            nc.scalar.activation(ln24[:, 16:21], vals[:, 16:21],
                                 mybir.ActivationFunctionType.Ln,
                                 scale=-1.0, bias=qn2_all[:],
                                 accum_out=s2[:])
            S = pp.tile([128, 1], F32, tag="S", name="S")
            denom = pp.tile([128, 1], F32, tag="denom", name="denom")
            lid = pp.tile([128, 1], BF16, tag="lid", name="lid")
            nc.vector.tensor_tensor(S[:], s1[:], s2[:], op=ADD)
            nc.vector.tensor_scalar(denom[:], ln24[:, 20:21], -20.0, S[:],
                                    op0=MULT, op1=ADD)
            # lid = 1/denom (bf16); the -2K and regression weights live in
            # wmask, applied by a single block-diagonal matmul
            with nc.allow_low_precision(reason="lid bf16 feeds bf16 matmul"):
                nc.vector.reciprocal(lid[:], denom[:])
            if _dbg:
                nc.sync.dma_start(out=dbg_vals[:], in_=vals[:])
                nc.sync.dma_start(out=dbg_lid[:], in_=lid[:])
                nc.sync.dma_start(out=dbg_qn2[:], in_=qn2_all[:])

            # ---- regression via one block-diagonal matmul + sigmoid
            lps = ps.tile([B, 1], F32, tag="lps", name="lps")
            nc.tensor.matmul(lps[:], wmask[:], lid[:],
                             start=True, stop=True, tile_position=(0, 0))
            # sigmoid(x) = 1/(1+exp(-x)); wmask carries +2K*w so lps is
            # -logit_wo_b, and wb[4] holds -b: Exp stays in table set 6
            eneg = pp.tile([B, 1], F32, tag="eneg", name="eneg")
            nc.scalar.activation(eneg[:], lps[:],
                                 mybir.ActivationFunctionType.Exp,
                                 scale=1.0, bias=wbc[:, 4:5])
            res = pp.tile([B, 1], F32, tag="res", name="res")
            nc.vector.tensor_scalar_add(res[:], eneg[:], 1.0)
            nc.vector.reciprocal(res[:], res[:])
            nc.sync.dma_start(out=out[:], in_=res[:])

    nc.compile()
    return nc


_NC = None


def _get_nc():
    global _NC
    if _NC is None:
        _NC = build_nc()
    return _NC


def make_in_maps(inputs):
    feats = [np.asarray(inputs[f"feat{l}"], dtype=np.float32) for l in range(4)]
    refTs = [np.ascontiguousarray(
        np.asarray(inputs[f"ref{l}"], dtype=np.float32).T).astype(NP_F8)
        for l in range(4)]
    # centered, scaled ref square norms from the quantized tables
    # (consistent with what the device matmuls against)
    rn2c = np.zeros((4, R), np.float32)
    for l, (C, HW) in enumerate(LAYERS):
        rn2 = (refTs[l].astype(np.float32) ** 2).sum(axis=0)
        rn2c[l] = -(HW / 2.0) * (rn2 - C)
    rn2c = rn2c.reshape(1, 4 * R).astype(NP_BF16)
    # pack per-layer ref tables into the padded [128, nchunk, R] layout
    packed = []
    for l, (C, HW) in enumerate(LAYERS):
        nch = max(1, C // 128)
        p = np.zeros((128, nch, R), NP_F8)
        if l == 0:
            for w in range(4):
                p[32 * w:32 * w + 16, 0] = refTs[0][16 * w:16 * w + 16]
        else:
            for ci in range(nch):
                p[:, ci] = refTs[l][128 * ci:128 * ci + 128]
        packed.append(p)
    regw = np.asarray(inputs["reg_w"], dtype=np.float32).reshape(4)
    regb = np.asarray(inputs["reg_b"], dtype=np.float32).reshape(1)
    wb = np.concatenate([regw, regb]).reshape(1, 5).astype(np.float32)
    wmask = np.zeros((128, B), np.float32)
    for l in range(4):
        wmask[32 * l:32 * l + 32, :] = 2.0 * K * regw[l] * np.eye(B)
    wmask = wmask.astype(NP_BF16)
    wb[0, 4] = -wb[0, 4]
    assert int(inputs.get("k", K)) == K

    in_maps = []
    for c in range(N_CORES):
        m = {}
        for l, (C, HW) in enumerate(LAYERS):
            m[f"feat{l}"] = np.ascontiguousarray(
                feats[l][c * B:(c + 1) * B].reshape(B, C, HW)).astype(NP_F8)
            m[f"refT{l}"] = packed[l]
        m["rn2c"] = rn2c
        m["wb"] = wb
        m["wmask"] = wmask
        in_maps.append(m)
    return in_maps


def run(trace=False, **inputs):
    nc = _get_nc()
    in_maps = make_in_maps(inputs)
    res = run_bass_kernel_spmd(nc, in_maps, core_ids=list(range(N_CORES)),
                               trace=trace)
    full = np.empty((N_CORES * B,), dtype=np.float32)
    for c in range(N_CORES):
        full[c * B:(c + 1) * B] = res.results[c]["out"][:, 0]
    return full, res


def kernel(**inputs):
    return run(trace=False, **inputs)[0]



# revision 12
# speedup vs baseline: 1.1974x; 1.1974x over previous
"""LID detector kernel v5 for Trainium2 (8 NeuronCores, data-parallel batch).

v5 over v2: DVE pooling units use a single scalar_tensor_tensor with
accum_out per channel column (fold halves + free-dim sum in one op)
for even-width layers, replacing the pairwise fold trees; L1 tail
columns rebalanced from Act to the cheaper DVE path.


Layout per core (batch shard of 32 samples):
  - feat{l} DRAM [32, C, HW] fp8e4m3; DMA'd fully contiguous: partition
    p = 32*w + s holds channels [Cp*w : Cp*w+Cp] of sample s (Cp = C/4).
  - pooling: segment sums over HW via per-channel activation accumulate
    (Act), pairwise-fold trees (DVE), single tensor_reduce (Pool/gpsimd),
    into red2[l] [128, Cp] f32.
  - stream-transpose red2 -> pieces -> qT chunks [Cc, 32] bf16 (qT = sum
    over HW, i.e. HW*q).
  - distances, scaled by HW/2: ps_d[32l+s, j] =
      (HW*q).r + ones x rn2c  where rn2c = -(HW/2)(||r||^2 - C)
    accumulated on PE; d2~ = (HW/2)qn2 + (HW/2)C - ps_d via the Ln bias.
    (rn2c rows are host-precomputed from the bf16 ref tables, like the
    host-side refT transpose: one-time prep of the replicated constants.)
  - top-24 largest of ps_d (= smallest d2) straight from PSUM via
    3x max8 + 2x match_replace.
  - LID = -2k / sum(ln d2_i - ln d2_k)  (scale invariant);
    logit = w.lid + b -> sigmoid.
"""

import os
import sys

for _p in ("/opt/trn_rl_repo", "/root/.axon_site/_ro/trn_rl_repo"):
    if _p not in sys.path:
        sys.path.append(_p)

import numpy as np

import concourse.mybir as mybir
from concourse import bass, bacc
from concourse.dt import dt as cdt
from concourse.tile import TileContext
from concourse.bass_utils import run_bass_kernel_spmd

F32 = mybir.dt.float32
BF16 = mybir.dt.bfloat16
F8 = mybir.dt.float8e4
N_CORES = 8
B = 32
R = 2000
K = 20
LAYERS = [(64, 3136), (128, 784), (256, 196), (512, 49)]
NEG_BIG = -3.0e38
ADD = mybir.AluOpType.add
MULT = mybir.AluOpType.mult

NP_F8 = cdt.np(F8)
NP_BF16 = cdt.np(BF16)


def _rt_chunks(l):
    C = LAYERS[l][0]
    return [(i, min(128, C - i)) for i in range(0, C, 128)]


def _mm_chunks(l):
    # L0 is stored padded to 128 partitions (16 valid rows per 32-block,
    # zeros elsewhere), so its matmul contracts over all 128.
    return [(0, 128)] if l == 0 else _rt_chunks(l)


def _emit_tree(eng, v, scratch, nch, W, out_ap, use_stt=False):
    """Segment-sum v [P, nch, W] (fp8) into out_ap [P, nch] (f32) via
    pairwise folds through scratch [P, nch, W//2] (bf16).

    use_stt: emit scalar_tensor_tensor (a*1+b) instead of tensor_tensor —
    on gpsimd this avoids the low "Add" software-efficiency path."""
    def add(o, a, b):
        if use_stt:
            eng.scalar_tensor_tensor(o, a, 1.0, b, op0=MULT, op1=ADD)
        else:
            eng.tensor_tensor(o, a, b, op=ADD)

    h = W // 2
    add(scratch[:, :, 0:h], v[:, :, 0:h], v[:, :, h:2 * h])
    if W % 2:
        add(scratch[:, :, 0:1], scratch[:, :, 0:1], v[:, :, 2 * h:2 * h + 1])
    W = h
    while W > 2:
        if W % 2:
            add(scratch[:, :, 0:1], scratch[:, :, 0:1], scratch[:, :, W - 1:W])
            W -= 1
            continue
        h = W // 2
        add(scratch[:, :, 0:h], scratch[:, :, 0:h], scratch[:, :, h:W])
        W = h
    add(out_ap, scratch[:, :, 0], scratch[:, :, 1])


# stream plan: ordered events.
#  ("wb",)                         weights + rn2c DMAs + rn2 matmuls
#  ("rt", l)                       ref chunk DMAs for layer l
#  ("feat", l, c0, nch, splits)    feat chunk DMA + pooling units
#       splits: list of (eng, c0, nch) covering [c0, c0+nch)
#  ("finish", l[, half])           transpose + pieces + qsq/qn2 + qT matmuls
PLAN = [
    ("feat", 1, 0, 8, [("dve", 0, 8)]),
    ("feat", 0, 0, 2, [("act", 0, 2)]),
    ("wb",),
    ("feat", 0, 2, 2, [("act", 2, 2)]),
    ("feat", 1, 8, 8, [("dve", 8, 8)]),
    ("feat", 0, 4, 4, [("act", 4, 4)]),
    ("feat", 2, 0, 32, [("pool", 0, 32)]),
    ("feat", 3, 0, 128, [("dve", 0, 64), ("dve", 64, 64)]),
    ("rt", 3),
    ("finish", 3),
    ("feat", 0, 8, 4, [("act", 8, 2), ("dve", 10, 2)]),
    ("feat", 1, 16, 8, [("dve", 16, 5), ("act", 21, 3)]),
    ("feat", 2, 32, 32, [("pool", 32, 32)]),
    ("feat", 0, 12, 4, [("dve", 12, 2), ("dve", 14, 2)]),
    ("feat", 1, 24, 8, [("dve", 24, 4), ("act", 28, 4)]),
    ("rt", 2),
    ("rt", 0),
    ("rt", 1),
    ("warm",),
    ("finish", 2),
    ("finish", 0),
    ("finish", 1),
]


def build_nc():
    nc = bacc.Bacc("TRN2", target_bir_lowering=False, debug=False,
                   num_devices=N_CORES)

    feats = [nc.dram_tensor(f"feat{l}", [B, C, HW], F8, kind="ExternalInput")
             for l, (C, HW) in enumerate(LAYERS)]
    refTs = [nc.dram_tensor(f"refT{l}", [128, max(1, C // 128), R], F8,
                            kind="ExternalInput")
             for l, (C, _) in enumerate(LAYERS)]
    rn2c_d = nc.dram_tensor("rn2c", [1, 4 * R], BF16, kind="ExternalInput")
    wb = nc.dram_tensor("wb", [1, 5], F32, kind="ExternalInput")
    wmask_d = nc.dram_tensor("wmask", [128, B], BF16, kind="ExternalInput")
    out = nc.dram_tensor("out", [B, 1], F32, kind="ExternalOutput")
    _dbg = os.environ.get("DEBUG_LID") == "1"
    if _dbg:
        dbg_red = [nc.dram_tensor(f"dbg_red{l}", [128, C // 4], F32,
                                  kind="ExternalOutput")
                   for l, (C, _) in enumerate(LAYERS)]
        dbg_qt = nc.dram_tensor("dbg_qt1", [128, B], F32, kind="ExternalOutput")
        dbg_vals = nc.dram_tensor("dbg_vals", [128, 24], F32,
                                  kind="ExternalOutput")
        dbg_lid = nc.dram_tensor("dbg_lid", [128, 1], BF16, kind="ExternalOutput")
        dbg_qn2 = nc.dram_tensor("dbg_qn2", [128, 1], F32, kind="ExternalOutput")

    with TileContext(nc) as tc:
        with (
            tc.tile_pool(name="persist", bufs=1) as pp,
            tc.tile_pool(name="sq", bufs=2) as sqp,
            tc.tile_pool(name="scr_d", bufs=2) as scrd,
            tc.tile_pool(name="scr_p", bufs=2) as scrp,
            tc.tile_pool(name="tr", bufs=2) as trp,
            tc.tile_pool(name="psum", bufs=1, space=bass.MemorySpace.PSUM) as ps,
        ):
            # ---- persistent tiles
            ft = [pp.tile([128, C // 4, HW], F8, tag=f"ft{l}", name=f"ft{l}")
                  for l, (C, HW) in enumerate(LAYERS)]
            rtbig = [pp.tile([128, max(1, C // 128), R], F8, tag=f"rt{l}",
                             name=f"rt{l}")
                     for l, (C, _) in enumerate(LAYERS)]
            rt = {}
            for l, (C, _) in enumerate(LAYERS):
                for ci, (i, Cc) in enumerate(_mm_chunks(l)):
                    rt[(l, i)] = rtbig[l][:, ci, :]
            rn2c = pp.tile([1, 4 * R], BF16, tag="rn2c", name="rn2c")
            red2 = [pp.tile([128, max(32, C // 4)], F32, tag=f"red2_{l}",
                            name=f"red2_{l}")
                    for l, (C, _) in enumerate(LAYERS)]
            qT = {}
            for l, (C, _) in enumerate(LAYERS):
                for i, Cc in _mm_chunks(l):
                    qT[(l, i)] = pp.tile([Cc, B], BF16, tag=f"qT{l}_{i}",
                                         name=f"qT{l}_{i}")
            act_scratch = pp.tile([128, 3136], BF16, tag="act_scr",
                                  name="act_scr")
            ones1 = pp.tile([1, B], BF16, tag="ones1", name="ones1")
            ones_col = pp.tile([128, 1], BF16, tag="ones_col", name="ones_col")
            ones_row = pp.tile([1, B], F32, tag="ones_row", name="ones_row")
            wb_sb = pp.tile([1, 5], F32, tag="wb_sb", name="wb_sb")
            qn2_all = pp.tile([128, 1], F32, tag="qn2_all", name="qn2_all")
            vals = pp.tile([128, 24], F32, tag="vals", name="vals")
            wbc = pp.tile([B, 5], F32, tag="wbc", name="wbc")
            wmask = pp.tile([128, B], BF16, tag="wmask", name="wmask")
            warm = pp.tile([1, 2], F32, tag="warm", name="warm")

            # PSUM
            ps_d = ps.tile([128, R], F32, tag="ps_d", name="ps_d")
            warm_ps = ps.tile([16, 512], F32, tag="warm_ps", name="warm_ps")
            zeros16 = pp.tile([1, 16], BF16, tag="zeros16", name="zeros16")
            qn2ps = ps.tile([128, 1], F32, tag="qn2ps", name="qn2ps")
            wps = ps.tile([B, 5], F32, tag="wps", name="wps")

            nc.vector.memset(ones1[:], 1.0)
            nc.vector.memset(zeros16[:], 0.0)
            nc.vector.memset(ones_col[:], 1.0)
            nc.vector.memset(ones_row[:], 1.0)
            nc.vector.memset(red2[0][:, 16:32], 0.0)

            BLKS = [(b0, min(512, R - b0)) for b0 in range(0, R, 512)]
            nch_chain = {}    # (l, b0) -> matmuls remaining until stop
            qsq_pending = []  # layers whose qn2 work is deferred past fins

            def emit_wb():
                # pre-load activation table set 6 (ln+exp+copy+square): the
                # whole pipeline stays in one set, no loads on the tail
                nc.scalar.add_instruction(mybir.InstLoadActFuncSet(
                    name=nc.get_next_instruction_name(), ins=[], outs=[],
                    act_func_set_id=6))
                nc.sync.dma_start(out=wb_sb[:], in_=wb[:])
                nc.sync.dma_start(out=wmask[:], in_=wmask_d[:])
                nc.sync.dma_start(out=rn2c[:], in_=rn2c_d[:])
                # broadcast w/b to all partitions right away (tail shortcut)
                nc.tensor.matmul(wps[:], ones_row[:], wb_sb[:],
                                 start=True, stop=True)
                nc.scalar.copy(wbc[:], wps[:])
                # open every accumulation chain with the rn2c row
                for l in range(4):
                    nq = len(_mm_chunks(l))
                    for b0, nb in BLKS:
                        nch_chain[(l, b0)] = nq
                        nc.tensor.matmul(
                            ps_d[32 * l:32 * l + 32, b0:b0 + nb],
                            ones1[:], rn2c[0:1, l * R + b0:l * R + b0 + nb],
                            start=True, stop=False,
                            tile_position=(0, 32 * l),
                            skip_group_check=True)

            def emit_rt(l):
                nc.sync.dma_start(out=rtbig[l][:], in_=refTs[l][:])

            def emit_pool_unit(l, eng_name, c0, nch):
                C, HW = LAYERS[l]
                v = ft[l][:, c0:c0 + nch, :]
                dst = red2[l][:, c0:c0 + nch]
                if eng_name == "act":
                    for j in range(nch):
                        nc.scalar.activation(
                            act_scratch[:, 0:HW], ft[l][:, c0 + j, :],
                            mybir.ActivationFunctionType.Copy,
                            accum_out=red2[l][:, c0 + j:c0 + j + 1])
                elif eng_name == "pool":
                    scratch = scrp.tile([128, nch, HW // 2], BF16,
                                        tag="scr_pool",
                                        name=f"scr_pool_{l}_{c0}")
                    _emit_tree(nc.gpsimd, v, scratch, nch, HW, dst)
                elif HW % 2 == 0:
                    # one stt-accum per column: fold halves + free-dim sum
                    # in a single DVE op (validated on walrus)
                    h = HW // 2
                    for j in range(nch):
                        scr = scrd.tile([128, h], BF16, tag="scr_dve",
                                        name=f"scr_dve_{l}_{c0 + j}")
                        nc.vector.scalar_tensor_tensor(
                            scr[:, 0:h], ft[l][:, c0 + j, 0:h], 1.0,
                            ft[l][:, c0 + j, h:2 * h], op0=MULT, op1=ADD,
                            accum_out=red2[l][:, c0 + j:c0 + j + 1])
                else:
                    scratch = scrd.tile([128, nch, HW // 2], BF16,
                                        tag="scr_dve",
                                        name=f"scr_dve_{l}_{c0}")
                    _emit_tree(nc.vector, v, scratch, nch, HW, dst)

            def emit_qT_mms(l, ksub=None):
                chunks = _mm_chunks(l)
                for ti, (i, Cc) in enumerate(chunks):
                    k0, k1 = (0, Cc) if ksub is None else ksub
                    for b0, nb in BLKS:
                        nch_chain[(l, b0)] -= 1
                        nc.tensor.matmul(
                            ps_d[32 * l:32 * l + 32, b0:b0 + nb],
                            qT[(l, i)][k0:k1, :],
                            rt[(l, i)][k0:k1, b0:b0 + nb],
                            start=False, stop=(nch_chain[(l, b0)] == 0),
                            tile_position=(k0, 32 * l),
                            skip_group_check=True)

            def emit_qsq(l, i, k0, k1, first, last):
                qsq = sqp.tile([128, B], BF16, tag="qsq", name="qsq")
                C, HW = LAYERS[l]
                # qT holds HW*q; want (HW/2)*q^2 = Square(qT/sqrt(2 HW))
                n = k1 - k0
                nc.scalar.activation(qsq[0:n, :], qT[(l, i)][k0:k1, :],
                                     mybir.ActivationFunctionType.Square,
                                     scale=float(1.0 / np.sqrt(2.0 * HW)))
                nc.tensor.matmul(qn2ps[32 * l:32 * l + 32, 0:1],
                                 qsq[0:n, :], ones_col[0:n, 0:1],
                                 start=first, stop=last,
                                 tile_position=(0, 32 * l),
                                 skip_group_check=True)
                if last:
                    nc.scalar.activation(qn2_all[32 * l:32 * l + 32, :],
                                         qn2ps[32 * l:32 * l + 32, :],
                                         mybir.ActivationFunctionType.Copy,
                                         bias=float(HW / 2.0 * C))

            def emit_finish(l, half=None):
                C, HW = LAYERS[l]
                Cp = C // 4
                cols = (0, max(32, Cp))
                w_ = cols[1] - cols[0]
                tr = trp.tile([128, 128], F32, tag="tr", name=f"tr{l}{half}")
                nc.vector.transpose(tr[:, 0:w_], red2[l][:, cols[0]:cols[1]])
                if l == 0:
                    nc.vector.tensor_copy(qT[(0, 0)][:], tr[:, 0:32])
                else:
                    for w in range(4):
                        for v in range(w_ // 32):
                            cglob = Cp * w + cols[0] + 32 * v
                            if cglob >= C:
                                continue
                            i = (cglob // 128) * 128
                            row = cglob - i
                            dst = qT[(l, i)][row:row + 32, :]
                            piece = tr[32 * w:32 * w + 32, 32 * v:32 * v + 32]
                            nc.vector.tensor_copy(dst, piece)
                emit_qT_mms(l)
                qsq_pending.append(l)

            def emit_warm():
                # hold PE p-state through the close cascade: chained dummy
                # matmuls (zero weights) gated on the late rt1 DMA
                for i in range(14):
                    nc.tensor.matmul(warm_ps[0:16, :],
                                     zeros16[0:1, 0:16],
                                     rtbig[1][0:1, 0, 0:512],
                                     start=True, stop=True,
                                     tile_position=(0, 0),
                                     skip_group_check=True)

            for ev in PLAN:
                if ev[0] == "warm":
                    emit_warm()
                elif ev[0] == "wb":
                    emit_wb()
                elif ev[0] == "rt":
                    emit_rt(ev[1])
                elif ev[0] == "feat":
                    _, l, c0, nch, splits = ev
                    C, HW = LAYERS[l]
                    Cp = C // 4
                    src = bass.AP(
                        feats[l], c0 * HW,
                        [[Cp * HW, 4], [C * HW, B], [1, nch * HW]])
                    nc.sync.dma_start(out=ft[l][:, c0:c0 + nch, :], in_=src)
                    for eng_name, ec0, ench in splits:
                        emit_pool_unit(l, eng_name, ec0, ench)
                elif ev[0] == "finish":
                    emit_finish(ev[1], ev[2] if len(ev) > 2 else None)

            # deferred qn2 chains: overlap with topk (only the Ln needs them)
            for l in qsq_pending:
                chunks = _mm_chunks(l)
                for ti, (i, Cc) in enumerate(chunks):
                    emit_qsq(l, i, 0, Cc, ti == 0, ti == len(chunks) - 1)

            if _dbg:
                for l in range(4):
                    C = LAYERS[l][0]
                    nc.sync.dma_start(out=dbg_red[l][:],
                                      in_=red2[l][:, 0:C // 4])
                dq = pp.tile([128, B], F32, tag="dq", name="dq")
                nc.vector.tensor_copy(dq[:], qT[(1, 0)][:])
                nc.sync.dma_start(out=dbg_qt[:], in_=dq[:])

            # ---- top-24 (largest ps_d == smallest d2), straight from PSUM
            ln24 = pp.tile([128, 24], F32, tag="ln24", name="ln24")
            s1 = pp.tile([128, 1], F32, tag="s1", name="s1")
            s2 = pp.tile([128, 1], F32, tag="s2", name="s2")
            nc.vector.max(vals[:, 0:8], ps_d[:])
            nc.vector.match_replace(ps_d[:], vals[:, 0:8], ps_d[:], NEG_BIG)
            nc.vector.max(vals[:, 8:16], ps_d[:])
            # ln of ranks 1..15 overlaps the remaining top-k passes (Act
            # idle); accum_out folds the partial S-reduction in for free
            nc.scalar.activation(ln24[:, 1:16], vals[:, 1:16],
                                 mybir.ActivationFunctionType.Ln,
                                 scale=-1.0, bias=qn2_all[:],
                                 accum_out=s1[:])
            nc.vector.match_replace(ps_d[:], vals[:, 8:16], ps_d[:], NEG_BIG)
            nc.vector.max(vals[:, 16:24], ps_d[:])

            # ---- LID: d2~ = (HW/2)qn2 + (HW/2)C - vals; scale cancels in LID
            nc.scalar.activation(ln24[:, 16:21], vals[:, 16:21],
                                 mybir.ActivationFunctionType.Ln,
                                 scale=-1.0, bias=qn2_all[:],
                                 accum_out=s2[:])
            S = pp.tile([128, 1], F32, tag="S", name="S")
            denom = pp.tile([128, 1], F32, tag="denom", name="denom")
            lid = pp.tile([128, 1], BF16, tag="lid", name="lid")
            nc.vector.tensor_tensor(S[:], s1[:], s2[:], op=ADD)
            nc.vector.tensor_scalar(denom[:], ln24[:, 20:21], -20.0, S[:],
                                    op0=MULT, op1=ADD)
            # lid = 1/denom (bf16); the -2K and regression weights live in
            # wmask, applied by a single block-diagonal matmul
            with nc.allow_low_precision(reason="lid bf16 feeds bf16 matmul"):
                nc.vector.reciprocal(lid[:], denom[:])
            if _dbg:
                nc.sync.dma_start(out=dbg_vals[:], in_=vals[:])
                nc.sync.dma_start(out=dbg_lid[:], in_=lid[:])
                nc.sync.dma_start(out=dbg_qn2[:], in_=qn2_all[:])

            # ---- regression via one block-diagonal matmul + sigmoid
            lps = ps.tile([B, 1], F32, tag="lps", name="lps")
            nc.tensor.matmul(lps[:], wmask[:], lid[:],
                             start=True, stop=True, tile_position=(0, 0))
            # sigmoid(x) = 1/(1+exp(-x)); wmask carries +2K*w so lps is
            # -logit_wo_b, and wb[4] holds -b: Exp stays in table set 6
            eneg = pp.tile([B, 1], F32, tag="eneg", name="eneg")
            nc.scalar.activation(eneg[:], lps[:],
                                 mybir.ActivationFunctionType.Exp,
                                 scale=1.0, bias=wbc[:, 4:5])
            res = pp.tile([B, 1], F32, tag="res", name="res")
            nc.vector.tensor_scalar_add(res[:], eneg[:], 1.0)
            nc.vector.reciprocal(res[:], res[:])
            nc.sync.dma_start(out=out[:], in_=res[:])

    nc.compile()
    return nc


_NC = None


def _get_nc():
    global _NC
    if _NC is None:
        _NC = build_nc()
    return _NC


def make_in_maps(inputs):
    feats = [np.asarray(inputs[f"feat{l}"], dtype=np.float32) for l in range(4)]
    refTs = [np.ascontiguousarray(
        np.asarray(inputs[f"ref{l}"], dtype=np.float32).T).astype(NP_F8)
        for l in range(4)]
    # centered, scaled ref square norms from the quantized tables
    # (consistent with what the device matmuls against)
    rn2c = np.zeros((4, R), np.float32)
    for l, (C, HW) in enumerate(LAYERS):
        rn2 = (refTs[l].astype(np.float32) ** 2).sum(axis=0)
        rn2c[l] = -(HW / 2.0) * (rn2 - C)
    rn2c = rn2c.reshape(1, 4 * R).astype(NP_BF16)
    # pack per-layer ref tables into the padded [128, nchunk, R] layout
    packed = []
    for l, (C, HW) in enumerate(LAYERS):
        nch = max(1, C // 128)
        p = np.zeros((128, nch, R), NP_F8)
        if l == 0:
            for w in range(4):
                p[32 * w:32 * w + 16, 0] = refTs[0][16 * w:16 * w + 16]
        else:
            for ci in range(nch):
                p[:, ci] = refTs[l][128 * ci:128 * ci + 128]
        packed.append(p)
    regw = np.asarray(inputs["reg_w"], dtype=np.float32).reshape(4)
    regb = np.asarray(inputs["reg_b"], dtype=np.float32).reshape(1)
    wb = np.concatenate([regw, regb]).reshape(1, 5).astype(np.float32)
    wmask = np.zeros((128, B), np.float32)
    for l in range(4):
        wmask[32 * l:32 * l + 32, :] = 2.0 * K * regw[l] * np.eye(B)
    wmask = wmask.astype(NP_BF16)
    wb[0, 4] = -wb[0, 4]
    assert int(inputs.get("k", K)) == K

    in_maps = []
    for c in range(N_CORES):
        m = {}
        for l, (C, HW) in enumerate(LAYERS):
            m[f"feat{l}"] = np.ascontiguousarray(
                feats[l][c * B:(c + 1) * B].reshape(B, C, HW)).astype(NP_F8)
            m[f"refT{l}"] = packed[l]
        m["rn2c"] = rn2c
        m["wb"] = wb
        m["wmask"] = wmask
        in_maps.append(m)
    return in_maps


def run(trace=False, **inputs):
    nc = _get_nc()
    in_maps = make_in_maps(inputs)
    res = run_bass_kernel_spmd(nc, in_maps, core_ids=list(range(N_CORES)),
                               trace=trace)
    full = np.empty((N_CORES * B,), dtype=np.float32)
    for c in range(N_CORES):
        full[c * B:(c + 1) * B] = res.results[c]["out"][:, 0]
    return full, res


def kernel(**inputs):
    return run(trace=False, **inputs)[0]



# revision 13
# speedup vs baseline: 1.2050x; 1.0064x over previous
"""LID detector kernel v5 for Trainium2 (8 NeuronCores, data-parallel batch).

v5 over v2: DVE pooling units use a single scalar_tensor_tensor with
accum_out per channel column (fold halves + free-dim sum in one op)
for even-width layers, replacing the pairwise fold trees; L1 tail
columns rebalanced from Act to the cheaper DVE path.


Layout per core (batch shard of 32 samples):
  - feat{l} DRAM [32, C, HW] fp8e4m3; DMA'd fully contiguous: partition
    p = 32*w + s holds channels [Cp*w : Cp*w+Cp] of sample s (Cp = C/4).
  - pooling: segment sums over HW via per-channel activation accumulate
    (Act), pairwise-fold trees (DVE), single tensor_reduce (Pool/gpsimd),
    into red2[l] [128, Cp] f32.
  - stream-transpose red2 -> pieces -> qT chunks [Cc, 32] bf16 (qT = sum
    over HW, i.e. HW*q).
  - distances, scaled by HW/2: ps_d[32l+s, j] =
      (HW*q).r + ones x rn2c  where rn2c = -(HW/2)(||r||^2 - C)
    accumulated on PE; d2~ = (HW/2)qn2 + (HW/2)C - ps_d via the Ln bias.
    (rn2c rows are host-precomputed from the bf16 ref tables, like the
    host-side refT transpose: one-time prep of the replicated constants.)
  - top-24 largest of ps_d (= smallest d2) straight from PSUM via
    3x max8 + 2x match_replace.
  - LID = -2k / sum(ln d2_i - ln d2_k)  (scale invariant);
    logit = w.lid + b -> sigmoid.
"""

import os
import sys

for _p in ("/opt/trn_rl_repo", "/root/.axon_site/_ro/trn_rl_repo"):
    if _p not in sys.path:
        sys.path.append(_p)

import numpy as np

import concourse.mybir as mybir
from concourse import bass, bacc
from concourse.dt import dt as cdt
from concourse.tile import TileContext
from concourse.bass_utils import run_bass_kernel_spmd

F32 = mybir.dt.float32
BF16 = mybir.dt.bfloat16
F8 = mybir.dt.float8e4
N_CORES = 8
B = 32
R = 2000
K = 20
LAYERS = [(64, 3136), (128, 784), (256, 196), (512, 49)]
NEG_BIG = -3.0e38
ADD = mybir.AluOpType.add
MULT = mybir.AluOpType.mult

NP_F8 = cdt.np(F8)
NP_BF16 = cdt.np(BF16)


def _rt_chunks(l):
    C = LAYERS[l][0]
    return [(i, min(128, C - i)) for i in range(0, C, 128)]


def _mm_chunks(l):
    # L0 is stored padded to 128 partitions (16 valid rows per 32-block,
    # zeros elsewhere), so its matmul contracts over all 128.
    return [(0, 128)] if l == 0 else _rt_chunks(l)


def _emit_tree(eng, v, scratch, nch, W, out_ap, use_stt=False):
    """Segment-sum v [P, nch, W] (fp8) into out_ap [P, nch] (f32) via
    pairwise folds through scratch [P, nch, W//2] (bf16).

    use_stt: emit scalar_tensor_tensor (a*1+b) instead of tensor_tensor —
    on gpsimd this avoids the low "Add" software-efficiency path."""
    def add(o, a, b):
        if use_stt:
            eng.scalar_tensor_tensor(o, a, 1.0, b, op0=MULT, op1=ADD)
        else:
            eng.tensor_tensor(o, a, b, op=ADD)

    h = W // 2
    add(scratch[:, :, 0:h], v[:, :, 0:h], v[:, :, h:2 * h])
    if W % 2:
        add(scratch[:, :, 0:1], scratch[:, :, 0:1], v[:, :, 2 * h:2 * h + 1])
    W = h
    while W > 2:
        if W % 2:
            add(scratch[:, :, 0:1], scratch[:, :, 0:1], scratch[:, :, W - 1:W])
            W -= 1
            continue
        h = W // 2
        add(scratch[:, :, 0:h], scratch[:, :, 0:h], scratch[:, :, h:W])
        W = h
    add(out_ap, scratch[:, :, 0], scratch[:, :, 1])


# stream plan: ordered events.
#  ("wb",)                         weights + rn2c DMAs + rn2 matmuls
#  ("rt", l)                       ref chunk DMAs for layer l
#  ("feat", l, c0, nch, splits)    feat chunk DMA + pooling units
#       splits: list of (eng, c0, nch) covering [c0, c0+nch)
#  ("finish", l[, half])           transpose + pieces + qsq/qn2 + qT matmuls
PLAN = [
    ("feat", 1, 0, 8, [("dve", 0, 8)]),
    ("feat", 0, 0, 2, [("act", 0, 2)]),
    ("wb",),
    ("feat", 0, 2, 2, [("act", 2, 2)]),
    ("feat", 1, 8, 8, [("dve", 8, 8)]),
    ("feat", 0, 4, 4, [("act", 4, 4)]),
    ("feat", 2, 0, 32, [("pool", 0, 32)]),
    ("feat", 3, 0, 128, [("dve", 0, 64), ("dve", 64, 64)]),
    ("rt", 3),
    ("finish", 3),
    ("feat", 0, 8, 4, [("act", 8, 2), ("dve", 10, 2)]),
    ("feat", 1, 16, 8, [("dve", 16, 5), ("act", 21, 3)]),
    ("feat", 2, 32, 32, [("pool", 32, 32)]),
    ("feat", 0, 12, 4, [("dve", 12, 2), ("dve", 14, 2)]),
    ("feat", 1, 24, 8, [("dve", 24, 4), ("act", 28, 4)]),
    ("rt", 2),
    ("rt", 0),
    ("rt", 1),
    ("warm",),
    ("finish", 1),
    ("finish", 0),
    ("finish", 2),
]


def build_nc():
    nc = bacc.Bacc("TRN2", target_bir_lowering=False, debug=False,
                   num_devices=N_CORES)

    feats = [nc.dram_tensor(f"feat{l}", [B, C, HW], F8, kind="ExternalInput")
             for l, (C, HW) in enumerate(LAYERS)]
    refTs = [nc.dram_tensor(f"refT{l}", [128, max(1, C // 128), R], F8,
                            kind="ExternalInput")
             for l, (C, _) in enumerate(LAYERS)]
    rn2c_d = nc.dram_tensor("rn2c", [1, 4 * R], BF16, kind="ExternalInput")
    wb = nc.dram_tensor("wb", [1, 5], F32, kind="ExternalInput")
    wmask_d = nc.dram_tensor("wmask", [128, B], BF16, kind="ExternalInput")
    out = nc.dram_tensor("out", [B, 1], F32, kind="ExternalOutput")
    _dbg = os.environ.get("DEBUG_LID") == "1"
    if _dbg:
        dbg_red = [nc.dram_tensor(f"dbg_red{l}", [128, C // 4], F32,
                                  kind="ExternalOutput")
                   for l, (C, _) in enumerate(LAYERS)]
        dbg_qt = nc.dram_tensor("dbg_qt1", [128, B], F32, kind="ExternalOutput")
        dbg_vals = nc.dram_tensor("dbg_vals", [128, 24], F32,
                                  kind="ExternalOutput")
        dbg_lid = nc.dram_tensor("dbg_lid", [128, 1], BF16, kind="ExternalOutput")
        dbg_qn2 = nc.dram_tensor("dbg_qn2", [128, 1], F32, kind="ExternalOutput")

    with TileContext(nc) as tc:
        with (
            tc.tile_pool(name="persist", bufs=1) as pp,
            tc.tile_pool(name="sq", bufs=2) as sqp,
            tc.tile_pool(name="scr_d", bufs=2) as scrd,
            tc.tile_pool(name="scr_p", bufs=2) as scrp,
            tc.tile_pool(name="tr", bufs=2) as trp,
            tc.tile_pool(name="psum", bufs=1, space=bass.MemorySpace.PSUM) as ps,
        ):
            # ---- persistent tiles
            ft = [pp.tile([128, C // 4, HW], F8, tag=f"ft{l}", name=f"ft{l}")
                  for l, (C, HW) in enumerate(LAYERS)]
            rtbig = [pp.tile([128, max(1, C // 128), R], F8, tag=f"rt{l}",
                             name=f"rt{l}")
                     for l, (C, _) in enumerate(LAYERS)]
            rt = {}
            for l, (C, _) in enumerate(LAYERS):
                for ci, (i, Cc) in enumerate(_mm_chunks(l)):
                    rt[(l, i)] = rtbig[l][:, ci, :]
            rn2c = pp.tile([1, 4 * R], BF16, tag="rn2c", name="rn2c")
            red2 = [pp.tile([128, max(32, C // 4)], F32, tag=f"red2_{l}",
                            name=f"red2_{l}")
                    for l, (C, _) in enumerate(LAYERS)]
            qT = {}
            for l, (C, _) in enumerate(LAYERS):
                for i, Cc in _mm_chunks(l):
                    qT[(l, i)] = pp.tile([Cc, B], BF16, tag=f"qT{l}_{i}",
                                         name=f"qT{l}_{i}")
            act_scratch = pp.tile([128, 3136], BF16, tag="act_scr",
                                  name="act_scr")
            ones1 = pp.tile([1, B], BF16, tag="ones1", name="ones1")
            ones_col = pp.tile([128, 1], BF16, tag="ones_col", name="ones_col")
            ones_row = pp.tile([1, B], F32, tag="ones_row", name="ones_row")
            wb_sb = pp.tile([1, 5], F32, tag="wb_sb", name="wb_sb")
            qn2_all = pp.tile([128, 1], F32, tag="qn2_all", name="qn2_all")
            vals = pp.tile([128, 24], F32, tag="vals", name="vals")
            wbc = pp.tile([B, 5], F32, tag="wbc", name="wbc")
            wmask = pp.tile([128, B], BF16, tag="wmask", name="wmask")
            warm = pp.tile([1, 2], F32, tag="warm", name="warm")

            # PSUM
            ps_d = ps.tile([128, R], F32, tag="ps_d", name="ps_d")
            warm_ps = ps.tile([16, 512], F32, tag="warm_ps", name="warm_ps")
            zeros16 = pp.tile([1, 16], BF16, tag="zeros16", name="zeros16")
            qn2ps = ps.tile([128, 1], F32, tag="qn2ps", name="qn2ps")
            wps = ps.tile([B, 5], F32, tag="wps", name="wps")

            nc.vector.memset(ones1[:], 1.0)
            nc.vector.memset(zeros16[:], 0.0)
            nc.vector.memset(ones_col[:], 1.0)
            nc.vector.memset(ones_row[:], 1.0)
            nc.vector.memset(red2[0][:, 16:32], 0.0)

            BLKS = [(b0, min(512, R - b0)) for b0 in range(0, R, 512)]
            nch_chain = {}    # (l, b0) -> matmuls remaining until stop
            qsq_pending = []  # layers whose qn2 work is deferred past fins

            def emit_wb():
                # pre-load activation table set 6 (ln+exp+copy+square): the
                # whole pipeline stays in one set, no loads on the tail
                nc.scalar.add_instruction(mybir.InstLoadActFuncSet(
                    name=nc.get_next_instruction_name(), ins=[], outs=[],
                    act_func_set_id=6))
                nc.sync.dma_start(out=wb_sb[:], in_=wb[:])
                nc.sync.dma_start(out=wmask[:], in_=wmask_d[:])
                nc.sync.dma_start(out=rn2c[:], in_=rn2c_d[:])
                # broadcast w/b to all partitions right away (tail shortcut)
                nc.tensor.matmul(wps[:], ones_row[:], wb_sb[:],
                                 start=True, stop=True)
                nc.scalar.copy(wbc[:], wps[:])
                # open every accumulation chain with the rn2c row
                for l in range(4):
                    nq = len(_mm_chunks(l))
                    for b0, nb in BLKS:
                        nch_chain[(l, b0)] = nq
                        nc.tensor.matmul(
                            ps_d[32 * l:32 * l + 32, b0:b0 + nb],
                            ones1[:], rn2c[0:1, l * R + b0:l * R + b0 + nb],
                            start=True, stop=False,
                            tile_position=(0, 32 * l),
                            skip_group_check=True)

            def emit_rt(l):
                nc.sync.dma_start(out=rtbig[l][:], in_=refTs[l][:])

            def emit_pool_unit(l, eng_name, c0, nch):
                C, HW = LAYERS[l]
                v = ft[l][:, c0:c0 + nch, :]
                dst = red2[l][:, c0:c0 + nch]
                if eng_name == "act":
                    for j in range(nch):
                        nc.scalar.activation(
                            act_scratch[:, 0:HW], ft[l][:, c0 + j, :],
                            mybir.ActivationFunctionType.Copy,
                            accum_out=red2[l][:, c0 + j:c0 + j + 1])
                elif eng_name == "pool":
                    scratch = scrp.tile([128, nch, HW // 2], BF16,
                                        tag="scr_pool",
                                        name=f"scr_pool_{l}_{c0}")
                    _emit_tree(nc.gpsimd, v, scratch, nch, HW, dst)
                elif HW % 2 == 0:
                    # one stt-accum per column: fold halves + free-dim sum
                    # in a single DVE op (validated on walrus)
                    h = HW // 2
                    for j in range(nch):
                        scr = scrd.tile([128, h], BF16, tag="scr_dve",
                                        name=f"scr_dve_{l}_{c0 + j}")
                        nc.vector.scalar_tensor_tensor(
                            scr[:, 0:h], ft[l][:, c0 + j, 0:h], 1.0,
                            ft[l][:, c0 + j, h:2 * h], op0=MULT, op1=ADD,
                            accum_out=red2[l][:, c0 + j:c0 + j + 1])
                else:
                    scratch = scrd.tile([128, nch, HW // 2], BF16,
                                        tag="scr_dve",
                                        name=f"scr_dve_{l}_{c0}")
                    _emit_tree(nc.vector, v, scratch, nch, HW, dst)

            def emit_qT_mms(l, ksub=None):
                chunks = _mm_chunks(l)
                for ti, (i, Cc) in enumerate(chunks):
                    k0, k1 = (0, Cc) if ksub is None else ksub
                    for b0, nb in BLKS:
                        nch_chain[(l, b0)] -= 1
                        nc.tensor.matmul(
                            ps_d[32 * l:32 * l + 32, b0:b0 + nb],
                            qT[(l, i)][k0:k1, :],
                            rt[(l, i)][k0:k1, b0:b0 + nb],
                            start=False, stop=(nch_chain[(l, b0)] == 0),
                            tile_position=(k0, 32 * l),
                            skip_group_check=True)

            def emit_qsq(l, i, k0, k1, first, last):
                qsq = sqp.tile([128, B], BF16, tag="qsq", name="qsq")
                C, HW = LAYERS[l]
                # qT holds HW*q; want (HW/2)*q^2 = Square(qT/sqrt(2 HW))
                n = k1 - k0
                nc.scalar.activation(qsq[0:n, :], qT[(l, i)][k0:k1, :],
                                     mybir.ActivationFunctionType.Square,
                                     scale=float(1.0 / np.sqrt(2.0 * HW)))
                nc.tensor.matmul(qn2ps[32 * l:32 * l + 32, 0:1],
                                 qsq[0:n, :], ones_col[0:n, 0:1],
                                 start=first, stop=last,
                                 tile_position=(0, 32 * l),
                                 skip_group_check=True)
                if last:
                    nc.scalar.activation(qn2_all[32 * l:32 * l + 32, :],
                                         qn2ps[32 * l:32 * l + 32, :],
                                         mybir.ActivationFunctionType.Copy,
                                         bias=float(HW / 2.0 * C))

            def emit_finish(l, half=None):
                C, HW = LAYERS[l]
                Cp = C // 4
                cols = (0, max(32, Cp))
                w_ = cols[1] - cols[0]
                tr = trp.tile([128, 128], F32, tag="tr", name=f"tr{l}{half}")
                nc.vector.transpose(tr[:, 0:w_], red2[l][:, cols[0]:cols[1]])
                if l == 0:
                    nc.vector.tensor_copy(qT[(0, 0)][:], tr[:, 0:32])
                else:
                    for w in range(4):
                        for v in range(w_ // 32):
                            cglob = Cp * w + cols[0] + 32 * v
                            if cglob >= C:
                                continue
                            i = (cglob // 128) * 128
                            row = cglob - i
                            dst = qT[(l, i)][row:row + 32, :]
                            piece = tr[32 * w:32 * w + 32, 32 * v:32 * v + 32]
                            nc.vector.tensor_copy(dst, piece)
                emit_qT_mms(l)
                qsq_pending.append(l)

            def emit_warm():
                # hold PE p-state through the close cascade: chained dummy
                # matmuls (zero weights) gated on the late rt1 DMA
                for i in range(12):
                    nc.tensor.matmul(warm_ps[0:16, :],
                                     zeros16[0:1, 0:16],
                                     rtbig[1][0:1, 0, 0:512],
                                     start=True, stop=True,
                                     tile_position=(0, 0),
                                     skip_group_check=True)

            for ev in PLAN:
                if ev[0] == "warm":
                    emit_warm()
                elif ev[0] == "wb":
                    emit_wb()
                elif ev[0] == "rt":
                    emit_rt(ev[1])
                elif ev[0] == "feat":
                    _, l, c0, nch, splits = ev
                    C, HW = LAYERS[l]
                    Cp = C // 4
                    src = bass.AP(
                        feats[l], c0 * HW,
                        [[Cp * HW, 4], [C * HW, B], [1, nch * HW]])
                    nc.sync.dma_start(out=ft[l][:, c0:c0 + nch, :], in_=src)
                    for eng_name, ec0, ench in splits:
                        emit_pool_unit(l, eng_name, ec0, ench)
                elif ev[0] == "finish":
                    emit_finish(ev[1], ev[2] if len(ev) > 2 else None)

            # deferred qn2 chains: overlap with topk (only the Ln needs them)
            for l in qsq_pending:
                chunks = _mm_chunks(l)
                for ti, (i, Cc) in enumerate(chunks):
                    emit_qsq(l, i, 0, Cc, ti == 0, ti == len(chunks) - 1)

            if _dbg:
                for l in range(4):
                    C = LAYERS[l][0]
                    nc.sync.dma_start(out=dbg_red[l][:],
                                      in_=red2[l][:, 0:C // 4])
                dq = pp.tile([128, B], F32, tag="dq", name="dq")
                nc.vector.tensor_copy(dq[:], qT[(1, 0)][:])
                nc.sync.dma_start(out=dbg_qt[:], in_=dq[:])

            # ---- top-24 (largest ps_d == smallest d2), straight from PSUM
            ln24 = pp.tile([128, 24], F32, tag="ln24", name="ln24")
            s1 = pp.tile([128, 1], F32, tag="s1", name="s1")
            s2 = pp.tile([128, 1], F32, tag="s2", name="s2")
            nc.vector.max(vals[:, 0:8], ps_d[:])
            nc.vector.match_replace(ps_d[:], vals[:, 0:8], ps_d[:], NEG_BIG)
            nc.vector.max(vals[:, 8:16], ps_d[:])
            # ln of ranks 1..15 overlaps the remaining top-k passes (Act
            # idle); accum_out folds the partial S-reduction in for free
            nc.scalar.activation(ln24[:, 1:16], vals[:, 1:16],
                                 mybir.ActivationFunctionType.Ln,
                                 scale=-1.0, bias=qn2_all[:],
                                 accum_out=s1[:])
            nc.vector.match_replace(ps_d[:], vals[:, 8:16], ps_d[:], NEG_BIG)
            nc.vector.max(vals[:, 16:24], ps_d[:])

            # ---- LID: d2~ = (HW/2)qn2 + (HW/2)C - vals; scale cancels in LID
            nc.scalar.activation(ln24[:, 16:21], vals[:, 16:21],
                                 mybir.ActivationFunctionType.Ln,
                                 scale=-1.0, bias=qn2_all[:],
                                 accum_out=s2[:])
            S = pp.tile([128, 1], F32, tag="S", name="S")
            denom = pp.tile([128, 1], F32, tag="denom", name="denom")
            lid = pp.tile([128, 1], BF16, tag="lid", name="lid")
            nc.vector.tensor_tensor(S[:], s1[:], s2[:], op=ADD)
            nc.vector.tensor_scalar(denom[:], ln24[:, 20:21], -20.0, S[:],
                                    op0=MULT, op1=ADD)
            # lid = 1/denom (bf16); the -2K and regression weights live in
            # wmask, applied by a single block-diagonal matmul
            with nc.allow_low_precision(reason="lid bf16 feeds bf16 matmul"):
                nc.vector.reciprocal(lid[:], denom[:])
            if _dbg:
                nc.sync.dma_start(out=dbg_vals[:], in_=vals[:])
                nc.sync.dma_start(out=dbg_lid[:], in_=lid[:])
                nc.sync.dma_start(out=dbg_qn2[:], in_=qn2_all[:])

            # ---- regression via one block-diagonal matmul + sigmoid
            lps = ps.tile([B, 1], F32, tag="lps", name="lps")
            nc.tensor.matmul(lps[:], wmask[:], lid[:],
                             start=True, stop=True, tile_position=(0, 0))
            # sigmoid(x) = 1/(1+exp(-x)); wmask carries +2K*w so lps is
            # -logit_wo_b, and wb[4] holds -b: Exp stays in table set 6
            eneg = pp.tile([B, 1], F32, tag="eneg", name="eneg")
            nc.scalar.activation(eneg[:], lps[:],
                                 mybir.ActivationFunctionType.Exp,
                                 scale=1.0, bias=wbc[:, 4:5])
            res = pp.tile([B, 1], F32, tag="res", name="res")
            nc.vector.tensor_scalar_add(res[:], eneg[:], 1.0)
            nc.vector.reciprocal(res[:], res[:])
            nc.sync.dma_start(out=out[:], in_=res[:])

    nc.compile()
    return nc


_NC = None


def _get_nc():
    global _NC
    if _NC is None:
        _NC = build_nc()
    return _NC


def make_in_maps(inputs):
    feats = [np.asarray(inputs[f"feat{l}"], dtype=np.float32) for l in range(4)]
    refTs = [np.ascontiguousarray(
        np.asarray(inputs[f"ref{l}"], dtype=np.float32).T).astype(NP_F8)
        for l in range(4)]
    # centered, scaled ref square norms from the quantized tables
    # (consistent with what the device matmuls against)
    rn2c = np.zeros((4, R), np.float32)
    for l, (C, HW) in enumerate(LAYERS):
        rn2 = (refTs[l].astype(np.float32) ** 2).sum(axis=0)
        rn2c[l] = -(HW / 2.0) * (rn2 - C)
    rn2c = rn2c.reshape(1, 4 * R).astype(NP_BF16)
    # pack per-layer ref tables into the padded [128, nchunk, R] layout
    packed = []
    for l, (C, HW) in enumerate(LAYERS):
        nch = max(1, C // 128)
        p = np.zeros((128, nch, R), NP_F8)
        if l == 0:
            for w in range(4):
                p[32 * w:32 * w + 16, 0] = refTs[0][16 * w:16 * w + 16]
        else:
            for ci in range(nch):
                p[:, ci] = refTs[l][128 * ci:128 * ci + 128]
        packed.append(p)
    regw = np.asarray(inputs["reg_w"], dtype=np.float32).reshape(4)
    regb = np.asarray(inputs["reg_b"], dtype=np.float32).reshape(1)
    wb = np.concatenate([regw, regb]).reshape(1, 5).astype(np.float32)
    wmask = np.zeros((128, B), np.float32)
    for l in range(4):
        wmask[32 * l:32 * l + 32, :] = 2.0 * K * regw[l] * np.eye(B)
    wmask = wmask.astype(NP_BF16)
    wb[0, 4] = -wb[0, 4]
    assert int(inputs.get("k", K)) == K

    in_maps = []
    for c in range(N_CORES):
        m = {}
        for l, (C, HW) in enumerate(LAYERS):
            m[f"feat{l}"] = np.ascontiguousarray(
                feats[l][c * B:(c + 1) * B].reshape(B, C, HW)).astype(NP_F8)
            m[f"refT{l}"] = packed[l]
        m["rn2c"] = rn2c
        m["wb"] = wb
        m["wmask"] = wmask
        in_maps.append(m)
    return in_maps


def run(trace=False, **inputs):
    nc = _get_nc()
    in_maps = make_in_maps(inputs)
    res = run_bass_kernel_spmd(nc, in_maps, core_ids=list(range(N_CORES)),
                               trace=trace)
    full = np.empty((N_CORES * B,), dtype=np.float32)
    for c in range(N_CORES):
        full[c * B:(c + 1) * B] = res.results[c]["out"][:, 0]
    return full, res


def kernel(**inputs):
    return run(trace=False, **inputs)[0]

